# revision 1
# baseline (speedup 1.0000x reference)
"""Trainium2 Bass kernel for nn_MAB (dense transformer block).

Reference computation (B=32, N=512, D=512, H=8, dh=64):
    q = (Q @ Wq.T + bq)  k = (K @ Wk.T + bk)  v = (K @ Wv.T + bv)
    scores = einsum("bqhd,bkhd->bhqk", q, k) / sqrt(512)
    A = softmax(scores, axis=2)            # over the QUERY axis!
    attn = einsum("bhqk,bkhd->bqhd", A, v).reshape(B, N, D)
    out = Q + attn @ Wo.T + bo
    ffn = relu(out @ W1.T + b1) @ W2.T + b2
    return out + ffn

Strategy: pure data-parallel over batch: 8 cores x 4 batches, zero
collectives.  On-chip activations are kept in TRANSPOSED layout
([feature, token], feature on partitions) so every matmul contracts over
partitions without any on-chip transposes; host pre-transposes Q/K and
the weights, and re-transposes the output.  Matmuls run in float32r
(full PE rate at moving-dim >= 256, ~tf32 precision).

Softmax over the query axis is computed on scores^T tiles ([k, q],
q on the free axis): ACT exp with fused free-axis accumulation, then the
reciprocal row-sums are folded into v (64x fewer elements than A).
Attention runs per head-PAIR: the two heads of a pair occupy disjoint
row groups (scores, K=64) / col groups (attn-apply, M=64) of the PE
array via tile_position, so their matmuls execute concurrently.
E and v~ are bf16 (attn matmul at full rate; softmax tolerance is wide).
"""

import math
import os
import sys

import numpy as np

sys.path.insert(0, "/opt/trn_rl_repo")

import concourse.bass as bass  # noqa: E402
import concourse.tile as tile  # noqa: E402
from concourse import bacc  # noqa: E402
from concourse import mybir  # noqa: E402
from concourse.bass_utils import run_bass_kernel_spmd  # noqa: E402

F32 = mybir.dt.float32
F32R = mybir.dt.float32r
BF16 = mybir.dt.bfloat16
AF = mybir.ActivationFunctionType
ALU = mybir.AluOpType

B, N, D, H = 32, 512, 512, 8
DH = D // H  # 64
NCORES = 8
BLOC = B // NCORES  # 4 batches per core
SCALE = 1.0 / math.sqrt(512.0)
P = 128
KC = D // P  # 4 contraction chunks
MC = D // P  # 4 output-feature chunks

_CACHE = {}


def _build_program(with_bias):
    nc = bacc.Bacc("TRN2", target_bir_lowering=False, debug=False,
                   num_devices=NCORES)

    # DRAM I/O ------------------------------------------------------------
    qT_d = nc.dram_tensor("qT", [BLOC, D, N], F32R, kind="ExternalInput").ap()
    kT_d = nc.dram_tensor("kT", [BLOC, D, N], F32R, kind="ExternalInput").ap()
    w_d = {}
    for nm in ("wq", "wk", "wv", "wo", "w1", "w2"):
        w_d[nm] = nc.dram_tensor(nm, [D, D], F32R, kind="ExternalInput").ap()
    b_d = {}
    if with_bias:
        for nm in ("bq", "bk", "bv", "bo", "b1", "b2"):
            b_d[nm] = nc.dram_tensor(nm, [D], F32, kind="ExternalInput").ap()
    outT_d = nc.dram_tensor("outT", [BLOC, D, N], F32,
                            kind="ExternalOutput").ap()

    qT_v = qT_d.rearrange("b (o p) t -> b p o t", p=P)
    kT_v = kT_d.rearrange("b (o p) t -> b p o t", p=P)
    outT_v = outT_d.rearrange("b (o p) t -> b p o t", p=P)
    w_v = {k: v.rearrange("(o p) n -> p o n", p=P) for k, v in w_d.items()}
    b_v = {k: v.rearrange("(o p) -> p o", p=P) for k, v in b_d.items()}

    with tile.TileContext(nc) as tc:
        with (
            tc.tile_pool(name="weights", bufs=1) as wpool,
            tc.tile_pool(name="qin", bufs=3) as qin_pool,
            tc.tile_pool(name="kin", bufs=2) as kin_pool,
            tc.tile_pool(name="proj", bufs=3) as proj_pool,
            tc.tile_pool(name="exp", bufs=4) as exp_pool,
            tc.tile_pool(name="rsum", bufs=4) as rsum_pool,
            tc.tile_pool(name="attn", bufs=2) as attn_pool,
            tc.tile_pool(name="ffn", bufs=2) as ffn_pool,
            tc.tile_pool(name="h1p", bufs=2) as h1_pool,
            tc.tile_pool(name="fin", bufs=4) as fin_pool,
            tc.tile_pool(name="psA", bufs=3, space="PSUM") as psA,
            tc.tile_pool(name="psS", bufs=4, space="PSUM") as psS,
            tc.tile_pool(name="psB", bufs=1, space="PSUM") as psB,
        ):
            # ---- resident weights/biases --------------------------------
            w_sb = {}
            for nm in ("wq", "wk", "wv", "wo", "w1", "w2"):
                w_sb[nm] = wpool.tile([P, KC, D], F32R, tag=f"w_{nm}",
                                      name=f"w_{nm}")
            qt0 = qin_pool.tile([P, KC, N], F32R, tag="qt", name="qt0")
            kt0 = kin_pool.tile([P, KC, N], F32R, tag="kt", name="kt0")
            for kc in range(KC):
                nc.sync.dma_start(out=w_sb["wq"][:, kc, :], in_=w_v["wq"][:, kc, :])
                nc.sync.dma_start(out=qt0[:, kc, :], in_=qT_v[0][:, kc, :])
            for kc in range(KC):
                nc.sync.dma_start(out=w_sb["wk"][:, kc, :], in_=w_v["wk"][:, kc, :])
                nc.sync.dma_start(out=kt0[:, kc, :], in_=kT_v[0][:, kc, :])
            nc.sync.dma_start(out=w_sb["wv"][:], in_=w_v["wv"])
            b_sb = {}
            bv_bc = None
            if with_bias:
                for nm in ("bq", "bk", "bo", "b1", "b2"):
                    b_sb[nm] = wpool.tile([P, MC], F32, tag=f"b_{nm}",
                                          name=f"b_{nm}")
                    nc.sync.dma_start(out=b_sb[nm][:], in_=b_v[nm])
                bv_bc = wpool.tile([P, D], F32, tag="bv_bc")
                bv_src = bass.AP(tensor=b_d["bv"].tensor,
                                 offset=b_d["bv"].offset,
                                 ap=[[0, P], *b_d["bv"].ap])
                nc.sync.dma_start(out=bv_bc[:], in_=bv_src)

            def linearT(dst, rhs_src, wname, bias):
                """dst[:, m, :] ([P, MC, N] transposed layout) = W @ rhs + b"""
                for m in range(MC):
                    ps = psA.tile([P, N], F32, tag="psA")
                    for kc in range(KC):
                        nc.tensor.matmul(
                            ps, lhsT=w_sb[wname][:, kc, m * P:(m + 1) * P],
                            rhs=rhs_src[:, kc, :],
                            start=(kc == 0), stop=(kc == KC - 1))
                    if with_bias:
                        nc.vector.tensor_scalar(
                            out=dst[:, m, :], in0=ps,
                            scalar1=b_sb[bias][:, m:m + 1], scalar2=None,
                            op0=ALU.add)
                    else:
                        nc.vector.tensor_copy(out=dst[:, m, :], in_=ps)

            st = {}  # per-batch state tiles

            def emit_proj(b):
                if b == 0:
                    qt_b, kt_b = qt0, kt0
                else:
                    qt_b = qin_pool.tile([P, KC, N], F32R, tag="qt")
                    nc.sync.dma_start(out=qt_b[:], in_=qT_v[b])
                    kt_b = kin_pool.tile([P, KC, N], F32R, tag="kt")
                    nc.sync.dma_start(out=kt_b[:], in_=kT_v[b])

                qh = proj_pool.tile([P, MC, N], BF16, tag="qh")
                linearT(qh, qt_b, "wq", "bq")
                kh = proj_pool.tile([P, MC, N], BF16, tag="kh")
                linearT(kh, kt_b, "wk", "bk")

                v_b = proj_pool.tile([P, KC, D], BF16, tag="v")
                for tt in range(KC):
                    ps = psA.tile([P, D], F32, tag="psA")
                    for kc in range(KC):
                        nc.tensor.matmul(
                            ps, lhsT=kt_b[:, kc, tt * P:(tt + 1) * P],
                            rhs=w_sb["wv"][:, kc, :],
                            start=(kc == 0), stop=(kc == KC - 1))
                    if with_bias:
                        nc.vector.tensor_tensor(
                            out=v_b[:, tt, :], in0=ps, in1=bv_bc[:],
                            op=ALU.add)
                    else:
                        nc.vector.tensor_copy(out=v_b[:, tt, :], in_=ps)

                if b == 0:
                    # deferred weight loads: DMA overlaps attention of b=0
                    for nm in ("wo", "w1", "w2"):
                        nc.sync.dma_start(out=w_sb[nm][:], in_=w_v[nm])
                st[b] = {"qt": qt_b, "qh": qh, "kh": kh, "v": v_b}

            def emit_attention(b):
                qh, kh, v_b = st[b]["qh"], st[b]["kh"], st[b]["v"]
                # ---- attention, head pairs -----------------------------
                # pair hp = heads (2hp, 2hp+1): rows 0-63 / 64-127 of
                # feature chunk hp.  Scores row-packed (K=64 x2), attn
                # col-packed (M=64 x2) into one [128, N] psum.
                attnT = attn_pool.tile([P, MC, N], F32R, tag="attnT")
                for hp in range(MC):
                    e0 = exp_pool.tile([P, KC, N], BF16, tag="e", name="e0")
                    e1 = exp_pool.tile([P, KC, N], BF16, tag="e", name="e1")
                    racc = rsum_pool.tile([P, KC, 2], F32, tag="racc")
                    rrec = rsum_pool.tile([P, KC, 2], F32, tag="rrec")
                    vt0 = rsum_pool.tile([P, KC, DH], BF16, tag="vt",
                                         name="vt0")
                    vt1 = rsum_pool.tile([P, KC, DH], BF16, tag="vt",
                                         name="vt1")
                    ps = psB.tile([P, N], F32, tag="psB")
                    for j in range(KC):
                        js = slice(j * P, (j + 1) * P)
                        ps0 = psS.tile([P, N], F32, tag="psS")
                        nc.tensor.matmul(
                            ps0, lhsT=kh[0:DH, hp, js], rhs=qh[0:DH, hp, :],
                            start=True, stop=True)
                        ps1 = psS.tile([P, N], F32, tag="psS")
                        nc.tensor.matmul(
                            ps1, lhsT=kh[DH:P, hp, js], rhs=qh[DH:P, hp, :],
                            start=True, stop=True)
                        nc.scalar.activation(
                            out=e0[:, j, :], in_=ps0, func=AF.Exp,
                            scale=SCALE, accum_out=racc[:, j, 0:1])
                        nc.scalar.activation(
                            out=e1[:, j, :], in_=ps1, func=AF.Exp,
                            scale=SCALE, accum_out=racc[:, j, 1:2])
                        nc.vector.reciprocal(out=rrec[:, j, :],
                                             in_=racc[:, j, :])
                        nc.vector.tensor_tensor(
                            out=vt0[:, j, :],
                            in0=v_b[:, j, 2 * hp * DH:(2 * hp + 1) * DH],
                            in1=rrec[:, j, 0:1].to_broadcast((P, DH)),
                            op=ALU.mult)
                        nc.vector.tensor_tensor(
                            out=vt1[:, j, :],
                            in0=v_b[:, j, (2 * hp + 1) * DH:(2 * hp + 2) * DH],
                            in1=rrec[:, j, 1:2].to_broadcast((P, DH)),
                            op=ALU.mult)
                        nc.tensor.matmul(
                            ps[0:DH, :], lhsT=vt0[:, j, :], rhs=e0[:, j, :],
                            start=(j == 0), stop=(j == KC - 1),
                            tile_position=(0, 0))
                        nc.tensor.matmul(
                            ps[DH:P, :], lhsT=vt1[:, j, :], rhs=e1[:, j, :],
                            start=(j == 0), stop=(j == KC - 1),
                            tile_position=(0, DH))
                    nc.vector.tensor_copy(out=attnT[:, hp, :], in_=ps)
                st[b]["attnT"] = attnT

            def emit_ffn(b):
                attnT, qt_b = st[b]["attnT"], st[b]["qt"]
                # ---- out = Q + attn @ Wo.T + bo (transposed) -----------
                outT_b = ffn_pool.tile([P, MC, N], F32R, tag="outT")
                for m in range(MC):
                    ps = psA.tile([P, N], F32, tag="psA")
                    for kc in range(KC):
                        nc.tensor.matmul(
                            ps, lhsT=w_sb["wo"][:, kc, m * P:(m + 1) * P],
                            rhs=attnT[:, kc, :],
                            start=(kc == 0), stop=(kc == KC - 1))
                    if with_bias:
                        nc.vector.tensor_scalar(
                            out=outT_b[:, m, :], in0=ps,
                            scalar1=b_sb["bo"][:, m:m + 1], scalar2=None,
                            op0=ALU.add)
                        nc.vector.tensor_tensor(
                            out=outT_b[:, m, :], in0=outT_b[:, m, :],
                            in1=qt_b[:, m, :], op=ALU.add)
                    else:
                        nc.vector.tensor_tensor(
                            out=outT_b[:, m, :], in0=ps,
                            in1=qt_b[:, m, :], op=ALU.add)

                # ---- ffn h1 = relu(W1 out^T + b1) ----------------------
                h1 = h1_pool.tile([P, MC, N], F32R, tag="h1")
                for m in range(MC):
                    ps = psA.tile([P, N], F32, tag="psA")
                    for kc in range(KC):
                        nc.tensor.matmul(
                            ps, lhsT=w_sb["w1"][:, kc, m * P:(m + 1) * P],
                            rhs=outT_b[:, kc, :],
                            start=(kc == 0), stop=(kc == KC - 1))
                    nc.vector.tensor_scalar(
                        out=h1[:, m, :], in0=ps,
                        scalar1=b_sb["b1"][:, m:m + 1] if with_bias else 0.0,
                        scalar2=0.0,
                        op0=ALU.add, op1=ALU.max)

                # ---- final = out + W2 h1 + b2, DMA out -----------------
                for m in range(MC):
                    ps = psA.tile([P, N], F32, tag="psA")
                    for kc in range(KC):
                        nc.tensor.matmul(
                            ps, lhsT=w_sb["w2"][:, kc, m * P:(m + 1) * P],
                            rhs=h1[:, kc, :],
                            start=(kc == 0), stop=(kc == KC - 1))
                    fin = fin_pool.tile([P, N], F32, tag="fin")
                    if with_bias:
                        nc.scalar.activation(
                            out=fin[:], in_=ps, func=AF.Identity,
                            bias=b_sb["b2"][:, m:m + 1], scale=1.0)
                        nc.vector.tensor_tensor(
                            out=fin[:], in0=fin[:], in1=outT_b[:, m, :],
                            op=ALU.add)
                    else:
                        nc.vector.tensor_tensor(
                            out=fin[:], in0=ps, in1=outT_b[:, m, :],
                            op=ALU.add)
                    nc.sync.dma_start(out=outT_v[b][:, m, :], in_=fin[:])
                del st[b]

            # software-pipelined emission: proj(b) || attention(b-1)
            # || ffn(b-2) -- lets the scheduler statically interleave
            # PE-heavy projection/FFN work with the ACT-bound softmax.
            for step in range(BLOC + 2):
                if 1 <= step <= BLOC:
                    emit_attention(step - 1)
                if step < BLOC:
                    emit_proj(step)
                if step >= 2:
                    emit_ffn(step - 2)

    nc.compile()
    return nc


def kernel(Q, K, Wq, bq, Wk, bk, Wv, bv, Wo, bo, W1, b1, W2, b2):
    Q = np.asarray(Q, dtype=np.float32)
    K = np.asarray(K, dtype=np.float32)

    biases = {nm: np.asarray(v, np.float32) for nm, v in
              (("bq", bq), ("bk", bk), ("bv", bv),
               ("bo", bo), ("b1", b1), ("b2", b2))}
    with_bias = any(np.any(v) for v in biases.values())

    key = ("nc", with_bias)
    if key not in _CACHE:
        _CACHE[key] = _build_program(with_bias)
    nc = _CACHE[key]

    common = {
        "wq": np.ascontiguousarray(np.asarray(Wq, np.float32).T),
        "wk": np.ascontiguousarray(np.asarray(Wk, np.float32).T),
        "wv": np.ascontiguousarray(np.asarray(Wv, np.float32).T),
        "wo": np.ascontiguousarray(np.asarray(Wo, np.float32).T),
        "w1": np.ascontiguousarray(np.asarray(W1, np.float32).T),
        "w2": np.ascontiguousarray(np.asarray(W2, np.float32).T),
    }
    if with_bias:
        common.update(biases)
    in_maps = []
    for c in range(NCORES):
        sl = slice(c * BLOC, (c + 1) * BLOC)
        in_maps.append({
            "qT": np.ascontiguousarray(Q[sl].transpose(0, 2, 1)),
            "kT": np.ascontiguousarray(K[sl].transpose(0, 2, 1)),
            **common,
        })

    trace = bool(int(os.environ.get("KERNEL_TRACE", "0")))
    res = run_bass_kernel_spmd(nc, in_maps, core_ids=list(range(NCORES)),
                               trace=trace)
    if trace and res.exec_time_ns is not None:
        print(f"HW exec time: {res.exec_time_ns} ns")
        if res.instructions_and_trace is not None:
            print("trace:", res.instructions_and_trace[1])

    out = np.empty((B, N, D), np.float32)
    for c in range(NCORES):
        out[c * BLOC:(c + 1) * BLOC] = res.results[c]["outT"].transpose(0, 2, 1)
    return out



# revision 6
# speedup vs baseline: 1.0395x; 1.0395x over previous
"""Trainium2 Bass kernel for nn_MAB (dense transformer block).

Reference computation (B=32, N=512, D=512, H=8, dh=64):
    q = (Q @ Wq.T + bq)  k = (K @ Wk.T + bk)  v = (K @ Wv.T + bv)
    scores = einsum("bqhd,bkhd->bhqk", q, k) / sqrt(512)
    A = softmax(scores, axis=2)            # over the QUERY axis!
    attn = einsum("bhqk,bkhd->bqhd", A, v).reshape(B, N, D)
    out = Q + attn @ Wo.T + bo
    ffn = relu(out @ W1.T + b1) @ W2.T + b2
    return out + ffn

Strategy: pure data-parallel over batch: 8 cores x 4 batches, zero
collectives.  All activations are kept TRANSPOSED on-chip ([feature,
token]) so every matmul contracts over partitions.

Speed levers vs the f32r baseline:
  * q/k/v/o projections run as fp8e4m3 DoubleRow matmuls (256-deep
    contraction per instruction, 0.5 cyc/row): weights are host-quantized
    to fp8 with power-of-two scales, Q/K are host-quantized to fp8.
  * the attention apply also runs fp8 DoubleRow: E = exp(scores) is
    written by ACT directly as fp8, and vt = v * (512/rsum) (the x512
    keeps vt out of fp8-denormal territory; the 1/512 and the fp8 weight
    scales are folded into the exp bias / eviction scales).
  * softmax-over-q runs on scores^T tiles ([key, q]): ACT exp over
    two-bank PSUM pairs; the per-key row-sums come from either the fused
    ACT accumulator (one [128,512] exp per bank) or a DVE tensor_reduce,
    split by a static knob so ACT and DVE finish together.
  * residuals are folded into the matmul accumulations (identity-matmul
    rows) so PSUM evictions are plain copies that can be placed on either
    ACT or DVE; the vt scaling runs on the otherwise idle GPSIMD engine.
  * FFN matmuls stay bf16 (fp8 there breaks the 2e-2 error budget).
"""

import math
import os
import sys

import numpy as np

sys.path.insert(0, "/opt/trn_rl_repo")

import ml_dtypes  # noqa: E402

import concourse.bass as bass  # noqa: E402
import concourse.tile as tile  # noqa: E402
from concourse import bacc  # noqa: E402
from concourse import mybir  # noqa: E402
from concourse.bass_utils import run_bass_kernel_spmd  # noqa: E402

F32 = mybir.dt.float32
F8 = mybir.dt.float8e4
BF16 = mybir.dt.bfloat16
AF = mybir.ActivationFunctionType
ALU = mybir.AluOpType
DR = mybir.MatmulPerfMode.DoubleRow

B, N, D, H = 32, 512, 512, 8
DH = D // H  # 64
NCORES = 8
BLOC = B // NCORES  # 4 batches per core
SCALE = 1.0 / math.sqrt(512.0)
P = 128
KC = D // P  # 4 contraction chunks
MC = D // P  # 4 output-feature chunks

# engine-balance knobs (tuned against TimelineSim)
N_ACC = 55          # of 64 (h,b,t) exp-pairs: how many use ACT-accum rsum
EV_ACT = 0          # of the flexible evictions: every EV_ACT-th goes to ACT

_CACHE = {}


def _build_program(with_bias):
    nc = bacc.Bacc("TRN2", target_bir_lowering=False, debug=False,
                   num_devices=NCORES)

    # ---- DRAM I/O -------------------------------------------------------
    qt8_d = nc.dram_tensor("qt8", [BLOC, D, N], F8, kind="ExternalInput").ap()
    kt8_d = nc.dram_tensor("kt8", [BLOC, D, N], F8, kind="ExternalInput").ap()
    qtb_d = nc.dram_tensor("qtb", [BLOC, D, N], BF16,
                           kind="ExternalInput").ap()
    wq_d = nc.dram_tensor("wq8", [D, D], F8, kind="ExternalInput").ap()
    wk_d = nc.dram_tensor("wk8", [D, D], F8, kind="ExternalInput").ap()
    wv_d = nc.dram_tensor("wv8", [D, D], F8, kind="ExternalInput").ap()
    wo_d = nc.dram_tensor("wo8", [DH, MC, 2, D], F8,
                          kind="ExternalInput").ap()
    w1_d = nc.dram_tensor("w1b", [D, D], BF16, kind="ExternalInput").ap()
    w2_d = nc.dram_tensor("w2b", [D, D], BF16, kind="ExternalInput").ap()
    id_d = nc.dram_tensor("idm", [P, P], BF16, kind="ExternalInput").ap()
    # cst cols: 0: exp scale  SCALE/(swq*swk); 1: exp bias ln(swv/512);
    #           2: 1/swv (attnT evict); 3: 1/swo (outT evict)
    cst_d = nc.dram_tensor("cst", [P, 4], F32, kind="ExternalInput").ap()
    b_d = {}
    if with_bias:
        # host pre-scales: bqs = swq*bq, bks = swk*bk, bvs = swv*bv,
        # bos = swo*bo, b1/b2 raw.  All as [1, D] rows.
        for nm in ("bqs", "bks", "bvs", "bos", "b1r", "b2r"):
            b_d[nm] = nc.dram_tensor(nm, [1, D], BF16,
                                     kind="ExternalInput").ap()
    outT_d = nc.dram_tensor("outT", [BLOC, D, N], F32,
                            kind="ExternalOutput").ap()

    qt8_v = qt8_d.rearrange("b (o p) t -> b p o t", p=P)
    kt8_v = kt8_d.rearrange("b (o p) t -> b p o t", p=P)
    qtb_v = qtb_d.rearrange("b (o p) t -> b p o t", p=P)
    outT_v = outT_d.rearrange("b (o p) t -> b p o t", p=P)
    wq_v = wq_d.rearrange("(o p) n -> p o n", p=P)
    wk_v = wk_d.rearrange("(o p) n -> p o n", p=P)
    wv_v = wv_d.rearrange("(o p) n -> p o n", p=P)
    w1_v = w1_d.rearrange("(o p) n -> p o n", p=P)
    w2_v = w2_d.rearrange("(o p) n -> p o n", p=P)

    # static round-robin schedulers for the balance knobs
    acc_ctr = [0]

    def use_accum():
        i = acc_ctr[0]
        acc_ctr[0] += 1
        return (i * N_ACC) % 64 < N_ACC

    ev_ctr = [0]

    def evict_engine():
        ev_ctr[0] += 1
        if EV_ACT and ev_ctr[0] % EV_ACT == 0:
            return "act"
        return "dve"

    with tile.TileContext(nc) as tc:
        with (
            tc.tile_pool(name="wpool", bufs=1) as wpool,
            tc.tile_pool(name="qin", bufs=2) as qin_pool,
            tc.tile_pool(name="kin", bufs=2) as kin_pool,
            tc.tile_pool(name="qbin", bufs=2) as qbin_pool,
            tc.tile_pool(name="proj", bufs=2) as proj_pool,
            tc.tile_pool(name="epool", bufs=3) as e_pool,
            tc.tile_pool(name="rpool", bufs=4) as r_pool,
            tc.tile_pool(name="vtp", bufs=3) as vt_pool,
            tc.tile_pool(name="atp", bufs=2) as at_pool,
            tc.tile_pool(name="outp", bufs=2) as out_pool,
            tc.tile_pool(name="h1p", bufs=2) as h1_pool,
            tc.tile_pool(name="finp", bufs=4) as fin_pool,
            tc.tile_pool(name="psS", bufs=2, space="PSUM") as psS,
            tc.tile_pool(name="psB", bufs=1, space="PSUM") as psB,
            tc.tile_pool(name="psA", bufs=2, space="PSUM") as psA,
        ):
            # ---- resident weights / constants ---------------------------
            wq_sb = wpool.tile([P, KC, D], F8, tag="wq")
            wk_sb = wpool.tile([P, KC, D], F8, tag="wk")
            wv_sb = wpool.tile([P, KC, D], F8, tag="wv")
            wo_sb = wpool.tile([DH, MC, 2, D], F8, tag="wo")
            w1_sb = wpool.tile([P, KC, D], BF16, tag="w1")
            w2_sb = wpool.tile([P, KC, D], BF16, tag="w2")
            id_sb = wpool.tile([P, P], BF16, tag="idm")
            cst = wpool.tile([P, 4], F32, tag="cst")
            nc.sync.dma_start(out=cst[:], in_=cst_d)
            nc.sync.dma_start(out=id_sb[:], in_=id_d)
            for kc in range(KC):
                nc.sync.dma_start(out=wq_sb[:, kc, :], in_=wq_v[:, kc, :])
                nc.sync.dma_start(out=wk_sb[:, kc, :], in_=wk_v[:, kc, :])
                nc.sync.dma_start(out=wv_sb[:, kc, :], in_=wv_v[:, kc, :])
            nc.sync.dma_start(out=wo_sb[:], in_=wo_d)
            b_sb = {}
            ones_sb = None
            if with_bias:
                ones_sb = wpool.tile([1, N], BF16, tag="ones")
                nc.vector.memset(ones_sb[:], 1.0)
                for nm in b_d:
                    b_sb[nm] = wpool.tile([1, D], BF16, tag=f"b_{nm}")
                    nc.sync.dma_start(out=b_sb[nm][:], in_=b_d[nm])

            exp_scale = cst[:, 0:1]
            exp_bias = cst[:, 1:2]
            at_scale = cst[0:DH, 2:3]
            out_scale = cst[:, 3:4]

            def evict_copy(dst, src):
                if evict_engine() == "act":
                    nc.scalar.activation(out=dst, in_=src, func=AF.Identity,
                                         scale=1.0)
                else:
                    nc.vector.tensor_copy(out=dst, in_=src)

            def evict_scale(dst, src, scale_ap):
                if evict_engine() == "act":
                    nc.scalar.activation(out=dst, in_=src, func=AF.Identity,
                                         scale=scale_ap)
                else:
                    nc.vector.tensor_scalar(out=dst, in0=src,
                                            scalar1=scale_ap, scalar2=None,
                                            op0=ALU.mult)

            def evict_relu(dst, src):
                if evict_engine() == "act":
                    nc.scalar.activation(out=dst, in_=src, func=AF.Relu,
                                         scale=1.0)
                else:
                    nc.vector.tensor_scalar(out=dst, in0=src, scalar1=0.0,
                                            scalar2=0.0, op0=ALU.add,
                                            op1=ALU.max)

            st = {}  # per-batch live tiles

            def emit_proj(b):
                qt8 = qin_pool.tile([P, KC, N], F8, tag="qt8")
                kt8 = kin_pool.tile([P, KC, N], F8, tag="kt8")
                qtb = qbin_pool.tile([P, KC, N], BF16, tag="qtb")
                for kc in range(KC):
                    nc.sync.dma_start(out=qt8[:, kc, :], in_=qt8_v[b][:, kc, :])
                    nc.sync.dma_start(out=kt8[:, kc, :], in_=kt8_v[b][:, kc, :])
                    nc.sync.dma_start(out=qtb[:, kc, :], in_=qtb_v[b][:, kc, :])

                qh = proj_pool.tile([P, MC, N], BF16, tag="qh")
                kh = proj_pool.tile([P, MC, N], BF16, tag="kh")
                vh = proj_pool.tile([P, KC, N], BF16, tag="vh")
                # q/k projections: fp8 DoubleRow, 2 mms per output chunk
                for dst, w_sb, rhs8, bias in (
                    (qh, wq_sb, qt8, "bqs"), (kh, wk_sb, kt8, "bks"),
                ):
                    for m in range(MC):
                        ps = psA.tile([P, N], F32, tag="psA")
                        for kp in range(2):
                            nc.tensor.matmul(
                                ps, lhsT=w_sb[:, 2 * kp:2 * kp + 2,
                                              m * P:(m + 1) * P],
                                rhs=rhs8[:, 2 * kp:2 * kp + 2, :],
                                start=(kp == 0),
                                stop=(kp == 1 and not with_bias),
                                perf_mode=DR)
                        if with_bias:
                            nc.tensor.matmul(
                                ps, lhsT=b_sb[bias][:, m * P:(m + 1) * P],
                                rhs=ones_sb[:], start=False, stop=True)
                        evict_copy(dst[:, m, :], ps)
                # v projection (transposed: [key, feat]): lhsT = K^T chunk
                for tt in range(KC):
                    ps = psA.tile([P, N], F32, tag="psA")
                    for kp in range(2):
                        nc.tensor.matmul(
                            ps, lhsT=kt8[:, 2 * kp:2 * kp + 2,
                                         tt * P:(tt + 1) * P],
                            rhs=wv_sb[:, 2 * kp:2 * kp + 2, :],
                            start=(kp == 0),
                            stop=(kp == 1 and not with_bias),
                            perf_mode=DR)
                    if with_bias:
                        nc.tensor.matmul(ps, lhsT=ones_sb[:, 0:P],
                                         rhs=b_sb["bvs"][:],
                                         start=False, stop=True)
                    evict_copy(vh[:, tt, :], ps)
                st[b] = {"qh": qh, "kh": kh, "vh": vh, "qtb": qtb}

            def emit_attn(b):
                qh, kh, vh = st[b]["qh"], st[b]["kh"], st[b]["vh"]
                at = at_pool.tile([DH, MC, 2, N], F8, tag="at")
                for hp in range(MC):
                    psb = psB.tile([DH, 2, N], F32, tag="psB")
                    for hh in range(2):
                        h = 2 * hp + hh
                        r0, r1 = hh * DH, (hh + 1) * DH
                        e_t = e_pool.tile([P, KC, N], F8, tag="e")
                        rs = r_pool.tile([P, KC], F32, tag="rs")
                        rr = r_pool.tile([P, KC], F32, tag="rr")
                        vt = vt_pool.tile([P, KC, DH], F8, tag="vt")
                        for t in range(2):
                            ps = psS.tile([P, 2, N], F32, tag="psS")
                            for u in range(2):
                                j = 2 * t + u
                                nc.tensor.matmul(
                                    ps[:, u, :],
                                    lhsT=kh[r0:r1, hp, j * P:(j + 1) * P],
                                    rhs=qh[r0:r1, hp, :],
                                    start=True, stop=True)
                            if use_accum():
                                for u in range(2):
                                    j = 2 * t + u
                                    nc.scalar.activation(
                                        out=e_t[:, j, :], in_=ps[:, u, :],
                                        func=AF.Exp, scale=exp_scale,
                                        bias=exp_bias,
                                        accum_out=rs[:, j:j + 1])
                            else:
                                nc.scalar.activation(
                                    out=e_t[:, 2 * t:2 * t + 2, :], in_=ps[:],
                                    func=AF.Exp, scale=exp_scale,
                                    bias=exp_bias)
                                for u in range(2):
                                    j = 2 * t + u
                                    nc.vector.tensor_reduce(
                                        out=rs[:, j:j + 1], in_=e_t[:, j, :],
                                        axis=mybir.AxisListType.X, op=ALU.add)
                        nc.vector.reciprocal(out=rr[:], in_=rs[:])
                        nc.gpsimd.tensor_tensor(
                            out=vt[:], in0=vh[:, :, h * DH:(h + 1) * DH],
                            in1=rr[:, :, None].to_broadcast((P, KC, DH)),
                            op=ALU.mult)
                        for t in range(2):
                            nc.tensor.matmul(
                                psb[:, hh, :],
                                lhsT=vt[:, 2 * t:2 * t + 2, :],
                                rhs=e_t[:, 2 * t:2 * t + 2, :],
                                start=(t == 0), stop=(t == 1),
                                perf_mode=DR)
                    evict_scale(at[:, hp, :, :], psb[:], at_scale)
                st[b]["at"] = at

            def emit_oproj(b):
                at, qtb = st[b]["at"], st[b]["qtb"]
                outT = out_pool.tile([P, MC, N], BF16, tag="outT")
                for m in range(MC):
                    ps = psA.tile([P, N], F32, tag="psA")
                    for hp in range(MC):
                        nc.tensor.matmul(
                            ps, lhsT=wo_sb[:, hp, :, m * P:(m + 1) * P],
                            rhs=at[:, hp, :, :],
                            start=(hp == 0),
                            stop=(hp == MC - 1 and not with_bias),
                            perf_mode=DR)
                    if with_bias:
                        nc.tensor.matmul(
                            ps, lhsT=b_sb["bos"][:, m * P:(m + 1) * P],
                            rhs=ones_sb[:], start=False, stop=True)
                    # outT = ps/swo + Q^T  (STT, DVE only)
                    nc.vector.scalar_tensor_tensor(
                        out=outT[:, m, :], in0=ps, scalar=out_scale,
                        in1=qtb[:, m, :], op0=ALU.mult, op1=ALU.add)
                st[b]["outT"] = outT

            def emit_ffn(b):
                outT = st[b]["outT"]
                h1 = h1_pool.tile([P, MC, N], BF16, tag="h1")
                for m in range(MC):
                    ps = psA.tile([P, N], F32, tag="psA")
                    for kc in range(KC):
                        nc.tensor.matmul(
                            ps, lhsT=w1_sb[:, kc, m * P:(m + 1) * P],
                            rhs=outT[:, kc, :],
                            start=(kc == 0),
                            stop=(kc == KC - 1 and not with_bias))
                    if with_bias:
                        nc.tensor.matmul(
                            ps, lhsT=b_sb["b1r"][:, m * P:(m + 1) * P],
                            rhs=ones_sb[:], start=False, stop=True)
                    evict_relu(h1[:, m, :], ps)
                for m in range(MC):
                    ps = psA.tile([P, N], F32, tag="psA")
                    for kc in range(KC):
                        nc.tensor.matmul(
                            ps, lhsT=w2_sb[:, kc, m * P:(m + 1) * P],
                            rhs=h1[:, kc, :], start=(kc == 0), stop=False)
                    # residual fold: psum += I.T @ outT_m
                    nc.tensor.matmul(
                        ps, lhsT=id_sb[:], rhs=outT[:, m, :],
                        start=False, stop=(not with_bias))
                    if with_bias:
                        nc.tensor.matmul(
                            ps, lhsT=b_sb["b2r"][:, m * P:(m + 1) * P],
                            rhs=ones_sb[:], start=False, stop=True)
                    fin = fin_pool.tile([P, N], F32, tag="fin")
                    evict_copy(fin[:], ps)
                    nc.sync.dma_start(out=outT_v[b][:, m, :], in_=fin[:])
                del st[b]

            # deferred weight loads that overlap the first projections
            def load_late_weights():
                for kc in range(KC):
                    nc.sync.dma_start(out=w1_sb[:, kc, :], in_=w1_v[:, kc, :])
                    nc.sync.dma_start(out=w2_sb[:, kc, :], in_=w2_v[:, kc, :])

            # software pipeline: proj(b) || attn(b-1) || o+ffn(b-2)
            for step in range(BLOC + 2):
                if 1 <= step <= BLOC:
                    emit_attn(step - 1)
                if step < BLOC:
                    emit_proj(step)
                if step == 0:
                    load_late_weights()
                if step >= 2:
                    emit_oproj(step - 2)
                    emit_ffn(step - 2)

    nc.compile()
    return nc


def _pow2_scale(amax, target=64.0):
    if amax <= 0 or not np.isfinite(amax):
        return 1.0
    return float(2.0 ** round(math.log2(target / amax)))


def kernel(Q, K, Wq, bq, Wk, bk, Wv, bv, Wo, bo, W1, b1, W2, b2):
    Q = np.asarray(Q, dtype=np.float32)
    K = np.asarray(K, dtype=np.float32)
    Wq, Wk, Wv, Wo = (np.asarray(w, np.float32) for w in (Wq, Wk, Wv, Wo))
    W1, W2 = np.asarray(W1, np.float32), np.asarray(W2, np.float32)

    biases = {nm: np.asarray(v, np.float32) for nm, v in
              (("bq", bq), ("bk", bk), ("bv", bv),
               ("bo", bo), ("b1", b1), ("b2", b2))}
    with_bias = any(np.any(v) for v in biases.values())

    key = ("nc", with_bias)
    if key not in _CACHE:
        _CACHE[key] = _build_program(with_bias)
    nc = _CACHE[key]

    swq = _pow2_scale(np.abs(Wq).max())
    swk = _pow2_scale(np.abs(Wk).max())
    swv = _pow2_scale(np.abs(Wv).max())
    swo = _pow2_scale(np.abs(Wo).max())

    F8NP = ml_dtypes.float8_e4m3
    BFNP = ml_dtypes.bfloat16

    def w8T(W, s):
        return np.ascontiguousarray((W.T * s).astype(F8NP))

    # wo folded for 64-partition DoubleRow: [dh, hp, i, m]
    WoT = (Wo.T * swo).astype(F8NP)  # [feat_in, m]
    wo_f = np.ascontiguousarray(
        WoT.reshape(MC, 2, DH, D).transpose(2, 0, 1, 3)
    )  # wait: feat_in = (2hp+i)*64+p -> reshape (MC,2,DH,D) then (p,hp,i,m)

    cst = np.zeros((P, 4), np.float32)
    cst[:, 0] = SCALE / (swq * swk)
    cst[:, 1] = math.log(swv / 512.0)
    cst[:, 2] = 1.0 / swv
    cst[:, 3] = 1.0 / swo

    common = {
        "wq8": w8T(Wq, swq),
        "wk8": w8T(Wk, swk),
        "wv8": w8T(Wv, swv),
        "wo8": wo_f,
        "w1b": np.ascontiguousarray(W1.T.astype(BFNP)),
        "w2b": np.ascontiguousarray(W2.T.astype(BFNP)),
        "idm": np.eye(P, dtype=np.float32).astype(BFNP),
        "cst": cst,
    }
    if with_bias:
        common.update({
            "bqs": (biases["bq"] * swq).astype(BFNP)[None, :],
            "bks": (biases["bk"] * swk).astype(BFNP)[None, :],
            "bvs": (biases["bv"] * swv).astype(BFNP)[None, :],
            "bos": (biases["bo"] * swo).astype(BFNP)[None, :],
            "b1r": biases["b1"].astype(BFNP)[None, :],
            "b2r": biases["b2"].astype(BFNP)[None, :],
        })

    in_maps = []
    for c in range(NCORES):
        sl = slice(c * BLOC, (c + 1) * BLOC)
        qT = np.ascontiguousarray(Q[sl].transpose(0, 2, 1))
        kT = np.ascontiguousarray(K[sl].transpose(0, 2, 1))
        in_maps.append({
            "qt8": qT.astype(F8NP),
            "kt8": kT.astype(F8NP),
            "qtb": qT.astype(BFNP),
            **common,
        })

    trace = bool(int(os.environ.get("KERNEL_TRACE", "0")))
    res = run_bass_kernel_spmd(nc, in_maps, core_ids=list(range(NCORES)),
                               trace=trace)
    if trace and res.exec_time_ns is not None:
        print(f"HW exec time: {res.exec_time_ns} ns")

    out = np.empty((B, N, D), np.float32)
    for c in range(NCORES):
        out[c * BLOC:(c + 1) * BLOC] = res.results[c]["outT"].transpose(0, 2, 1)
    return out
